# revision 57
# baseline (speedup 1.0000x reference)
"""Canny NMS filter for 8x Trainium2 NeuronCores (Bass/Tile), fp32r edition.

Batch is sharded one image per core. Per core (img 3x1024x1024 -> 1024x1024):

  m   = (c0+c1+c2)            DMA-accumulate loads (SWDGE), fp32r tile
  mb  = gauss3x3 (x) pad(m)   banded fp32r matmuls on PE (1 cyc/row @512)
  gx/gy = sobel (x) pad(mb)   banded fp32r matmuls
  sqx -> mag (ACT Square), sqy (ACT Square), gys (ACT Copy), gxy (DVE)
  mag = sqx + DMA-accumulate(sqy)
  orientation (division-free, exact identity): with A=-tan(3pi/16),
  B=-tan(pi/16), P=A+B, Q=AB one has 1-Q = -P exactly, so
     q1*q2 = Q*mag^2 - (1-Q)^2 * gxy * (2*sqy - mag)
  and "not oriented" nm := [ gxy*(2*sqy-mag) <= lam*mag^2 ], lam = Q/(1-Q)^2.
  NMS: pul = conv(pad(mag), -dw3) - BIG*nm  (= ul - mag - BIG*nm; the -mag
  center tap is folded into the band), pdr likewise from -dw7.
     z1 = min(-K*pul, mag); z2 = min(-K*pdr, z1); out = max(z2, 0)
  which reproduces remove <=> oriented and max(ul,dr) >= mag, bit-correctly
  at the boundaries (exact ties included).

Row axis: 9 overlapping slabs of 128 partitions, 122 core rows each, 3 halo
rows per side; cross-partition work happens inside per-slab banded matmuls
whose band matrices (host-built from the input kernels) fold in
jnp.pad(mode='edge') clamping. Column axis: 3-col pads each side of SBUF
tiles, refreshed with edge values between stages.
"""

import math
import numpy as np

B, C, H, W = 8, 3, 1024, 1024
NCORES = 8
SLAB = 122
NSLABS = (H + SLAB - 1) // SLAB          # 9
PADL = 3                                  # col c stored at f = c + 3
FW = W + 2 * PADL                         # 1030
CHUNK = 512
BIG = np.float32(1e30)
BIGK = -1e30                              # scalar in the z-chain stt

A_ = -math.tan(3 * math.pi / 16)
B_ = -math.tan(math.pi / 16)
P_ = A_ + B_
Q_ = A_ * B_
LAM = Q_ / (1.0 - Q_) ** 2
SQLAM = math.sqrt(LAM)

_CACHE = {}


# ---------------------------------------------------------------------------
def _install_fixups():
    """This container's walrus encodes at most ONE sem wait per instruction
    (2 for EventSemaphore); the bass/tile build attaches more. Two patches:
    the TileContext tail drain (waits on every proc's clock) is split into a
    chain of single-wait sync nops, and a post-schedule pass moves excess
    waits from any instruction onto injected same-engine NoOps."""
    import concourse.tile as _tile
    from concourse.vector_clock import ScopedClock, VectorClock

    if getattr(_tile.TileContext, "_canny_patched", False):
        return

    def _drain_and_barrier(self, tick_clock, wait_clock):
        gcl = tick_clock.global_clock
        for i in range(len(gcl)):
            if gcl[i] == 0:
                continue
            vec = [0] * len(gcl)
            vec[i] = gcl[i]
            nop = self.nc.sync.nop(nofuse=True, hint="tail_drain_split")
            wait_clock.add_sem_waits(nop.ins,
                                     ScopedClock({None: VectorClock(vec)}))
        self.nc.sync.drain()
        self.nc.all_engine_barrier()
        assert self.sems is not None
        popped = self.nc._tile_sem_poison_stack.pop()
        assert popped is self._sem_poison
        self.nc.clear_and_free_semaphores(list(self.sems.allocated().values()))
        self.nc.all_engine_barrier()

    _tile.TileContext._drain_and_barrier = _drain_and_barrier
    _tile.TileContext._canny_patched = True


def _split_excess_waits(nc):
    import concourse.mybir as mybir
    for fn in nc.m.functions:
        for blk in fn.blocks:
            insts = list(blk.instructions)
            out, changed = [], False
            for inst in insts:
                si = inst.sync_info
                cap = 2 if isinstance(inst, mybir.InstEventSemaphore) else 1
                if si is not None and si.on_wait and len(si.on_wait) > cap:
                    waits = list(si.on_wait)
                    for j, wt in enumerate(waits[cap:]):
                        nop = mybir.InstNoOp(name=f"{inst.name}-wsplit{j}")
                        nop.engine = inst.engine
                        nop.sync_info = mybir.SyncInfo(on_wait=[wt],
                                                       on_update=[])
                        out.append(nop)
                    si.on_wait = waits[:cap]
                    inst.sync_info = si
                    changed = True
                out.append(inst)
            if changed:
                blk.instructions = out


# ---------------------------------------------------------------------------
# host-side band-matrix construction
def _r0(s):
    return SLAB * s - PADL


def _band(s, taps, clamp):
    """lhsT[k, m]: out[m] = sum_j taps[j] * in[k(m, j)] for slab s.
    k(m, j) = m + j, optionally clamped (in partition space) to the image
    edge partitions; unclamped out-of-range taps are dropped (those output
    rows are never consumed)."""
    Wm = np.zeros((128, 128), np.float64)
    lo = PADL if (clamp and s == 0) else None
    hi = (H - 1 - _r0(s)) if (clamp and s == NSLABS - 1) else None
    for m in range(128):
        for off, cf in taps.items():
            k = m + off
            if lo is not None and k < lo:
                k = lo
            if hi is not None and k > hi:
                k = hi
            if 0 <= k < 128:
                Wm[k, m] += cf
    return Wm


def _col_taps(k3x3, dc):
    col = k3x3[:, dc]
    return {j - 1: col[j] for j in range(3)}


def _build_bands(gauss_w, sobel_x, sobel_y, dir_w):
    """Returns (wr f32 [nr,128,128] fp32r bands, wbig bf16 [128,128],
    index mapping (kind[, variant], dc) -> idx into wr)."""
    import ml_dtypes
    g = np.asarray(gauss_w, np.float64).reshape(3, 3) / 3.0
    sx = np.asarray(sobel_x, np.float64).reshape(3, 3)
    sy = np.asarray(sobel_y, np.float64).reshape(3, 3)
    dw = np.asarray(dir_w, np.float64).reshape(8, 3, 3)

    wr, index, dedup = [], {}, {}

    def addr(key, mat64):
        m32 = np.asarray(mat64, np.float32)
        hb = m32.tobytes()
        if hb in dedup:
            index[key] = dedup[hb]
            return
        index[key] = dedup[hb] = len(wr)
        wr.append(m32)

    # blur: no clamping (input m already carries duplicated edge rows)
    for dc in range(3):
        addr(("blur", dc), _band(4, _col_taps(g, dc), clamp=False))
    # sobel first, then NMS shift bands (taps from -dw[ch] — the center -1
    # folds the "- mag" term), so the packed weight tensor can be DMA'd in
    # stage-sized pieces.
    for grp in ((("gx", sx), ("gy", sy)), (("ul", -dw[3]), ("dr", -dw[7]))):
        for v in range(3):
            s = {0: 0, 1: 4, 2: NSLABS - 1}[v]
            for nm_, kk in grp:
                for dc in range(3):
                    if not np.any(kk[:, dc]):
                        continue
                    addr((nm_, v, dc),
                         _band(s, _col_taps(kk, dc), clamp=True))

    wr = np.stack(wr).astype(np.float32)
    wbig = (-np.eye(128) * float(BIG)).astype(ml_dtypes.bfloat16)
    return wr, wbig, index


def _structure_key(index):
    return tuple(sorted(map(repr, index.items())))


# ---------------------------------------------------------------------------
def _build_module(index, nr):
    import concourse.bass as bass
    import concourse.tile as tile
    import concourse.mybir as mybir
    from contextlib import ExitStack

    F32 = mybir.dt.float32
    F32R = mybir.dt.float32r
    BF16 = mybir.dt.bfloat16
    AF = mybir.ActivationFunctionType
    Al = mybir.AluOpType

    nc = bass.Bass("TRN2", target_bir_lowering=False, debug=False,
                   num_devices=NCORES)
    img_d = nc.dram_tensor("img", [C, H, W], F32, kind="ExternalInput").ap()
    wr_d = nc.dram_tensor("wr", [128, nr * 128], F32R,
                          kind="ExternalInput").ap()
    wb_d = nc.dram_tensor("wb", [128, 128], BF16, kind="ExternalInput").ap()
    out_d = nc.dram_tensor("out", [H, W], F32, kind="ExternalOutput").ap()

    # weight-load split points: blur bands, then sobel bands, then NMS
    # bands, so slab-0 image loads and the first blur aren't stuck behind
    # one big weight DMA on the serial DMA device.
    n_blur = 1 + max(i for k, i in index.items() if k[0] == "blur")
    n_sob = 1 + max(i for k, i in index.items() if k[0] in ("gx", "gy"))

    def wm(key):
        i = index[key]
        return wrt[:, i * 128:(i + 1) * 128]

    def has(key):
        return key in index

    with tile.TileContext(nc) as tc, ExitStack() as ctx:
        wpool = ctx.enter_context(tc.tile_pool(name="wpool", bufs=1))
        mpool = ctx.enter_context(tc.tile_pool(name="mpool", bufs=5))
        stagep = ctx.enter_context(tc.tile_pool(name="stagep", bufs=1))
        mbp = ctx.enter_context(tc.tile_pool(name="mbp", bufs=3))
        magp = ctx.enter_context(tc.tile_pool(name="magp", bufs=3))
        tl = ctx.enter_context(tc.tile_pool(name="tl", bufs=4))
        ps_mb = ctx.enter_context(
            tc.tile_pool(name="ps_mb", bufs=2, space="PSUM"))
        ps_g = ctx.enter_context(
            tc.tile_pool(name="ps_g", bufs=4, space="PSUM"))
        ps_e = ctx.enter_context(
            tc.tile_pool(name="ps_e", bufs=2, space="PSUM"))

        wrt = wpool.tile([128, nr * 128], F32R, name="wrt")
        wbt = wpool.tile([128, 128], BF16, name="wbt")

        m_of = {}

        def emit_loads(sx, staged=False):
            """Load + channel-sum slab sx into a fresh m tile. Normally c0
            rides SP/HWDGE (plain write) and c1/c2 accumulate via Pool
            SWDGE; `staged` (ramp-up slabs) loads all channels via SP into
            a staging tile and sums on the then-idle DVE, skipping the
            DMA-accumulate dependency chain."""
            if sx >= NSLABS:
                return
            r0x = _r0(sx)
            p_lox = PADL if sx == 0 else 0
            p_hix = (H - 1 - r0x) if sx == NSLABS - 1 else 127
            if sx == 0:
                # ramp fast path (slab 0): keep the 3 channels separate
                # (plain SP loads, no c0->c1->c2 accumulate chain to wait
                # on) and let the blur matmuls sum them (blur-of-sum =
                # sum-of-blurs) on the startup-idle PE.
                mst = m_of[sx] = stagep.tile([128, 3 * FW], F32R,
                                             name="mst", tag="mst")
                for c in range(C):
                    base = c * FW
                    nc.sync.dma_start(
                        mst[p_lox:p_hix + 1, base + PADL:base + PADL + W],
                        img_d[c, r0x + p_lox:r0x + p_hix + 1, :]
                        .bitcast(F32R))
                    if sx == 0:
                        nc.sync.dma_start(
                            mst[PADL - 1:PADL, base + PADL:base + PADL + W],
                            img_d[c, 0:1, :].bitcast(F32R))
                return
            m = m_of[sx] = mpool.tile([128, FW], F32R, name="m", tag="m")
            if sx == NSLABS - 1:
                nc.vector.memset(m[64:128, :].bitcast(F32), 0.0)
            if staged:
                stg = stagep.tile([128, 3 * W], F32, name="stg", tag="stg")
                for c in range(C):
                    nc.sync.dma_start(
                        stg[p_lox:p_hix + 1, c * W:(c + 1) * W],
                        img_d[c, r0x + p_lox:r0x + p_hix + 1, :])
                nc.vector.tensor_tensor(stg[:, 0:W], stg[:, 0:W],
                                        stg[:, W:2 * W], Al.add)
                nc.vector.tensor_tensor(m[:, PADL:PADL + W], stg[:, 0:W],
                                        stg[:, 2 * W:3 * W], Al.add)
            else:
                if sx < 2:
                    nc.sync.dma_start(m[p_lox:p_hix + 1, PADL:PADL + W],
                                      img_d[0, r0x + p_lox:r0x + p_hix + 1,
                                            :].bitcast(F32R))
                else:
                    # later slabs' c0 rides the Pool queue so it cannot
                    # queue-jump slab 0/1's channel sums on the DMA device
                    nc.gpsimd.dma_start(
                        m[p_lox:p_hix + 1, PADL:PADL + W],
                        img_d[0, r0x + p_lox:r0x + p_hix + 1, :])
                for c in range(1, C):
                    nc.gpsimd.dma_start(
                        m[p_lox:p_hix + 1, PADL:PADL + W],
                        img_d[c, r0x + p_lox:r0x + p_hix + 1, :],
                        accum_op=Al.add)
            if sx == NSLABS - 1:             # duplicated bottom edge row
                nc.sync.dma_start(m[p_hix + 1:p_hix + 2, PADL:PADL + W],
                                  img_d[0, H - 1:H, :].bitcast(F32R))
                for c in range(1, C):
                    nc.gpsimd.dma_start(
                        m[p_hix + 1:p_hix + 2, PADL:PADL + W],
                        img_d[c, H - 1:H, :],
                        accum_op=Al.add)

        # prologue: tiny blur-weight DMA early on the ACT HWDGE queue;
        # slabs 0/1 loads; remaining weights behind them on the Pool queue
        # so slab 0's channel sums aren't stuck behind 4us of weights on
        # the FIFO DMA device.
        nc.scalar.dma_start(wrt[:, :n_blur * 128], wr_d[:, :n_blur * 128])
        emit_loads(0)
        emit_loads(1)
        nc.sync.dma_start(wrt[:, n_blur * 128:n_sob * 128],
                          wr_d[:, n_blur * 128:n_sob * 128])
        nc.gpsimd.dma_start(wrt[:, n_sob * 128:], wr_d[:, n_sob * 128:])
        nc.gpsimd.dma_start(wbt[:], wb_d)
        emit_loads(2)
        emit_loads(3)

        for s in range(NSLABS):
            v = 0 if s == 0 else (2 if s == NSLABS - 1 else 1)
            r0 = _r0(s)
            m = m_of.pop(s)
            bases = list(range(0, 3 * FW, FW)) if s == 0 else [0]

            # col edge pads (both columns in one strided op, on ACT)
            for base in bases:
                nc.scalar.activation(
                    m[:, base + PADL - 1:base + PADL + W + 1:W + 1],
                    m[:, base + PADL:base + PADL + W:W - 1].bitcast(F32),
                    AF.Copy)

            # ---- blur (fp32r matmuls straight off m; slab 0 sums its 3
            # separate channel blocks inside the accumulation group) ----
            mb = mbp.tile([128, FW], F32R, name="mb", tag="mb")
            for h in range(2):
                f0 = PADL + CHUNK * h
                pm = ps_mb.tile([128, CHUNK], F32, name="pm", tag="pm")
                mms = [(base, dc) for base in bases for dc in range(3)]
                for i, (base, dc) in enumerate(mms):
                    nc.tensor.matmul(
                        pm[:], wm(("blur", dc)),
                        m[:, base + f0 + dc - 1:base + f0 + dc - 1 + CHUNK],
                        start=(i == 0), stop=(i == len(mms) - 1))
                nc.scalar.activation(mb[:, f0:f0 + CHUNK], pm[:], AF.Copy)
            nc.scalar.activation(mb[:, PADL - 1:PADL + W + 1:W + 1],
                                 mb[:, PADL:PADL + W:W - 1].bitcast(F32),
                                 AF.Copy)

            # ---- sobel + squares ----
            mag = magp.tile([128, FW], F32R, name="mag", tag="mag")
            sqy = tl.tile([128, W], F32, name="sqy", tag="sqy")
            gys = tl.tile([128, W], F32, name="gys", tag="gys")
            gxy = tl.tile([128, W], F32, name="gxy", tag="gxy")
            for h in range(2):
                f0 = PADL + CHUNK * h
                w0 = CHUNK * h
                pgx = ps_g.tile([128, CHUNK], F32, name="pgx", tag="pg")
                mms = [("gx", v, dc) for dc in range(3) if has(("gx", v, dc))]
                for i, key in enumerate(mms):
                    dc = key[2]
                    nc.tensor.matmul(
                        pgx[:], wm(key),
                        mb[:, f0 + dc - 1:f0 + dc - 1 + CHUNK],
                        start=(i == 0), stop=(i == len(mms) - 1))
                pgy = ps_g.tile([128, CHUNK], F32, name="pgy", tag="pg")
                mms = [("gy", v, dc) for dc in range(3) if has(("gy", v, dc))]
                for i, key in enumerate(mms):
                    dc = key[2]
                    nc.tensor.matmul(
                        pgy[:], wm(key),
                        mb[:, f0 + dc - 1:f0 + dc - 1 + CHUNK],
                        start=(i == 0), stop=(i == len(mms) - 1))
                # evacs: sqx straight into mag; sqy; gy copy; gxy
                nc.scalar.activation(mag[:, f0:f0 + CHUNK], pgx[:], AF.Square)
                nc.scalar.activation(sqy[:, w0:w0 + CHUNK], pgy[:], AF.Square)
                nc.scalar.activation(gys[:, w0:w0 + CHUNK], pgy[:], AF.Copy)
                nc.vector.tensor_tensor(
                    gxy[:, w0:w0 + CHUNK], pgx[:], gys[:, w0:w0 + CHUNK],
                    Al.mult)
            # loads for slab s+4: deep lookahead so stores' fin-waits on
            # SP never starve upcoming loads
            emit_loads(s + 4)

            # mag = sqx + sqy in place on DVE: a DMA accumulate queues
            # behind bulk lookahead loads on the FIFO DMA device (+3-7us on
            # the critical chain); the inline add costs 1.1us
            nc.gpsimd.tensor_tensor(mag[:, PADL:PADL + CHUNK],
                                     mag[:, PADL:PADL + CHUNK].bitcast(F32),
                                     sqy[:, 0:CHUNK], Al.add)
            nc.vector.tensor_tensor(
                mag[:, PADL + CHUNK:PADL + W],
                mag[:, PADL + CHUNK:PADL + W].bitcast(F32),
                sqy[:, CHUNK:W], Al.add)
            nc.scalar.activation(mag[:, PADL - 1:PADL + W + 1:W + 1],
                                 mag[:, PADL:PADL + W:W - 1].bitcast(F32),
                                 AF.Copy)

            # ---- orientation: nm = [gxy*(2*sqy - mag) <= lam*mag^2] ----
            magc = mag[:, PADL:PADL + W].bitcast(F32)
            wq = tl.tile([128, W], F32, name="wq", tag="wq")
            cq = tl.tile([128, W], F32, name="cq", tag="cq")
            m2 = tl.tile([128, W], F32, name="m2", tag="m2")
            nm = tl.tile([128, W], BF16, name="nm", tag="nm")
            nc.vector.scalar_tensor_tensor(
                wq[:], sqy[:], 2.0, magc, Al.mult, Al.subtract)
            nc.gpsimd.tensor_tensor(cq[:], gxy[:], wq[:], Al.mult)
            nc.scalar.activation(m2[:], magc, AF.Square, scale=SQLAM)
            nc.vector.tensor_tensor(nm[:], cq[:], m2[:], Al.is_le)

            # ---- NMS: banded shifts (-mag folded), -BIG*nm, z-chain ----
            fin = tl.tile([128, W], F32, name="fin", tag="fin")
            for h in range(2):
                f0 = PADL + CHUNK * h
                w0 = CHUNK * h
                pul = ps_e.tile([128, CHUNK], F32, name="pul", tag="pe")
                mms = [("ul", v, dc) for dc in range(3)
                       if has(("ul", v, dc))]
                for i, key in enumerate(mms):
                    dc = key[2]
                    nc.tensor.matmul(
                        pul[:], wm(key),
                        mag[:, f0 + dc - 1:f0 + dc - 1 + CHUNK],
                        start=(i == 0), stop=False)
                nc.tensor.matmul(pul[:], wbt[:], nm[:, w0:w0 + CHUNK],
                                 start=False, stop=True)
                z1 = tl.tile([128, CHUNK], F32, name="z1", tag="z1")
                nc.vector.scalar_tensor_tensor(
                    z1[:], pul[:], BIGK, mag[:, f0:f0 + CHUNK].bitcast(F32),
                    Al.mult, Al.min)
                pdr = ps_e.tile([128, CHUNK], F32, name="pdr", tag="pe")
                mms = [("dr", v, dc) for dc in range(3)
                       if has(("dr", v, dc))]
                for i, key in enumerate(mms):
                    dc = key[2]
                    nc.tensor.matmul(
                        pdr[:], wm(key),
                        mag[:, f0 + dc - 1:f0 + dc - 1 + CHUNK],
                        start=(i == 0), stop=False)
                nc.tensor.matmul(pdr[:], wbt[:], nm[:, w0:w0 + CHUNK],
                                 start=False, stop=True)
                z2 = tl.tile([128, CHUNK], F32, name="z2", tag="z2")
                nc.vector.scalar_tensor_tensor(
                    z2[:], pdr[:], BIGK, z1[:], Al.mult, Al.min)
                nc.vector.tensor_scalar(
                    fin[:, w0:w0 + CHUNK], z2[:], 0.0, None, Al.max)

            # store on SP; the 4-slab load lookahead keeps c0(s+4..) from
            # being starved behind this store's fin wait.
            row_lo = SLAB * s
            row_hi = min(H - 1, row_lo + SLAB - 1)
            nc.sync.dma_start(out_d[row_lo:row_hi + 1, :],
                              fin[PADL:PADL + row_hi - row_lo + 1, :])

    _split_excess_waits(nc)
    return nc


# ---------------------------------------------------------------------------
def kernel(**inputs):
    _install_fixups()

    img = np.ascontiguousarray(np.asarray(inputs["img"], np.float32))
    gauss_w = np.asarray(inputs["gauss_w"], np.float32)
    sobel_x = np.asarray(inputs["sobel_x"], np.float32)
    sobel_y = np.asarray(inputs["sobel_y"], np.float32)
    dir_w = np.asarray(inputs["dir_w"], np.float32)

    wr, wbig, index = _build_bands(gauss_w, sobel_x, sobel_y, dir_w)
    nr = wr.shape[0]
    # pack bands side-by-side: [128, nr*128]
    wr_pack = np.ascontiguousarray(
        np.concatenate([wr[i] for i in range(nr)], axis=1))
    skey = (_structure_key(index), nr)
    if _CACHE.get("skey") != skey:
        _CACHE["nc"] = _build_module(index, nr)
        _CACHE["skey"] = skey
    nc = _CACHE["nc"]

    from concourse.bass_utils import run_bass_kernel_spmd
    import os
    wbig = np.ascontiguousarray(wbig)
    in_maps = [{"img": np.ascontiguousarray(img[b]), "wr": wr_pack,
                "wb": wbig} for b in range(B)]
    trace = bool(int(os.environ.get("CANNY_TRACE", "0")))
    res = run_bass_kernel_spmd(nc, in_maps, core_ids=list(range(NCORES)),
                               trace=trace)
    if res.exec_time_ns is not None:
        _CACHE["exec_time_ns"] = res.exec_time_ns
    if res.instructions_and_trace is not None:
        _CACHE["trace_path"] = res.instructions_and_trace[1]
    out = np.stack([res.results[b]["out"] for b in range(B)])[:, None]
    return out.astype(np.float32)


# revision 58
# speedup vs baseline: 1.0199x; 1.0199x over previous
"""Canny NMS filter for 8x Trainium2 NeuronCores (Bass/Tile), fp32r edition.

Batch is sharded one image per core. Per core (img 3x1024x1024 -> 1024x1024):

  m   = (c0+c1+c2)            DMA-accumulate loads (SWDGE), fp32r tile
  mb  = gauss3x3 (x) pad(m)   banded fp32r matmuls on PE (1 cyc/row @512)
  gx/gy = sobel (x) pad(mb)   banded fp32r matmuls
  sqx -> mag (ACT Square), sqy (ACT Square), gys (ACT Copy), gxy (DVE)
  mag = sqx + DMA-accumulate(sqy)
  orientation (division-free, exact identity): with A=-tan(3pi/16),
  B=-tan(pi/16), P=A+B, Q=AB one has 1-Q = -P exactly, so
     q1*q2 = Q*mag^2 - (1-Q)^2 * gxy * (2*sqy - mag)
  and "not oriented" nm := [ gxy*(2*sqy-mag) <= lam*mag^2 ], lam = Q/(1-Q)^2.
  NMS: pul = conv(pad(mag), -dw3) - BIG*nm  (= ul - mag - BIG*nm; the -mag
  center tap is folded into the band), pdr likewise from -dw7.
     z1 = min(-K*pul, mag); z2 = min(-K*pdr, z1); out = max(z2, 0)
  which reproduces remove <=> oriented and max(ul,dr) >= mag, bit-correctly
  at the boundaries (exact ties included).

Row axis: 9 overlapping slabs of 128 partitions, 122 core rows each, 3 halo
rows per side; cross-partition work happens inside per-slab banded matmuls
whose band matrices (host-built from the input kernels) fold in
jnp.pad(mode='edge') clamping. Column axis: 3-col pads each side of SBUF
tiles, refreshed with edge values between stages.
"""

import math
import numpy as np

B, C, H, W = 8, 3, 1024, 1024
NCORES = 8
SLAB = 122
NSLABS = (H + SLAB - 1) // SLAB          # 9
PADL = 3                                  # col c stored at f = c + 3
FW = W + 2 * PADL                         # 1030
CHUNK = 512
BIG = np.float32(1e30)
BIGK = -1e30                              # scalar in the z-chain stt

A_ = -math.tan(3 * math.pi / 16)
B_ = -math.tan(math.pi / 16)
P_ = A_ + B_
Q_ = A_ * B_
LAM = Q_ / (1.0 - Q_) ** 2
SQLAM = math.sqrt(LAM)

_CACHE = {}


# ---------------------------------------------------------------------------
def _install_fixups():
    """This container's walrus encodes at most ONE sem wait per instruction
    (2 for EventSemaphore); the bass/tile build attaches more. Two patches:
    the TileContext tail drain (waits on every proc's clock) is split into a
    chain of single-wait sync nops, and a post-schedule pass moves excess
    waits from any instruction onto injected same-engine NoOps."""
    import concourse.tile as _tile
    from concourse.vector_clock import ScopedClock, VectorClock

    if getattr(_tile.TileContext, "_canny_patched", False):
        return

    def _drain_and_barrier(self, tick_clock, wait_clock):
        gcl = tick_clock.global_clock
        for i in range(len(gcl)):
            if gcl[i] == 0:
                continue
            vec = [0] * len(gcl)
            vec[i] = gcl[i]
            nop = self.nc.sync.nop(nofuse=True, hint="tail_drain_split")
            wait_clock.add_sem_waits(nop.ins,
                                     ScopedClock({None: VectorClock(vec)}))
        self.nc.sync.drain()
        self.nc.all_engine_barrier()
        assert self.sems is not None
        popped = self.nc._tile_sem_poison_stack.pop()
        assert popped is self._sem_poison
        self.nc.clear_and_free_semaphores(list(self.sems.allocated().values()))
        self.nc.all_engine_barrier()

    _tile.TileContext._drain_and_barrier = _drain_and_barrier
    _tile.TileContext._canny_patched = True


def _split_excess_waits(nc):
    import concourse.mybir as mybir
    for fn in nc.m.functions:
        for blk in fn.blocks:
            insts = list(blk.instructions)
            out, changed = [], False
            for inst in insts:
                si = inst.sync_info
                cap = 2 if isinstance(inst, mybir.InstEventSemaphore) else 1
                if si is not None and si.on_wait and len(si.on_wait) > cap:
                    waits = list(si.on_wait)
                    for j, wt in enumerate(waits[cap:]):
                        nop = mybir.InstNoOp(name=f"{inst.name}-wsplit{j}")
                        nop.engine = inst.engine
                        nop.sync_info = mybir.SyncInfo(on_wait=[wt],
                                                       on_update=[])
                        out.append(nop)
                    si.on_wait = waits[:cap]
                    inst.sync_info = si
                    changed = True
                out.append(inst)
            if changed:
                blk.instructions = out


# ---------------------------------------------------------------------------
# host-side band-matrix construction
def _r0(s):
    return SLAB * s - PADL


def _band(s, taps, clamp):
    """lhsT[k, m]: out[m] = sum_j taps[j] * in[k(m, j)] for slab s.
    k(m, j) = m + j, optionally clamped (in partition space) to the image
    edge partitions; unclamped out-of-range taps are dropped (those output
    rows are never consumed)."""
    Wm = np.zeros((128, 128), np.float64)
    lo = PADL if (clamp and s == 0) else None
    hi = (H - 1 - _r0(s)) if (clamp and s == NSLABS - 1) else None
    for m in range(128):
        for off, cf in taps.items():
            k = m + off
            if lo is not None and k < lo:
                k = lo
            if hi is not None and k > hi:
                k = hi
            if 0 <= k < 128:
                Wm[k, m] += cf
    return Wm


def _col_taps(k3x3, dc):
    col = k3x3[:, dc]
    return {j - 1: col[j] for j in range(3)}


def _build_bands(gauss_w, sobel_x, sobel_y, dir_w):
    """Returns (wr f32 [nr,128,128] fp32r bands, wbig bf16 [128,128],
    index mapping (kind[, variant], dc) -> idx into wr)."""
    import ml_dtypes
    g = np.asarray(gauss_w, np.float64).reshape(3, 3) / 3.0
    sx = np.asarray(sobel_x, np.float64).reshape(3, 3)
    sy = np.asarray(sobel_y, np.float64).reshape(3, 3)
    dw = np.asarray(dir_w, np.float64).reshape(8, 3, 3)

    wr, index, dedup = [], {}, {}

    def addr(key, mat64):
        m32 = np.asarray(mat64, np.float32)
        hb = m32.tobytes()
        if hb in dedup:
            index[key] = dedup[hb]
            return
        index[key] = dedup[hb] = len(wr)
        wr.append(m32)

    # blur: no clamping (input m already carries duplicated edge rows)
    for dc in range(3):
        addr(("blur", dc), _band(4, _col_taps(g, dc), clamp=False))
    # sobel first, then NMS shift bands (taps from -dw[ch] — the center -1
    # folds the "- mag" term), so the packed weight tensor can be DMA'd in
    # stage-sized pieces.
    for grp in ((("gx", sx), ("gy", sy)), (("ul", -dw[3]), ("dr", -dw[7]))):
        for v in range(3):
            s = {0: 0, 1: 4, 2: NSLABS - 1}[v]
            for nm_, kk in grp:
                for dc in range(3):
                    if not np.any(kk[:, dc]):
                        continue
                    addr((nm_, v, dc),
                         _band(s, _col_taps(kk, dc), clamp=True))

    wr = np.stack(wr).astype(np.float32)
    wbig = (-np.eye(128) * float(BIG)).astype(ml_dtypes.bfloat16)
    return wr, wbig, index


def _structure_key(index):
    return tuple(sorted(map(repr, index.items())))


# ---------------------------------------------------------------------------
def _build_module(index, nr):
    import concourse.bass as bass
    import concourse.tile as tile
    import concourse.mybir as mybir
    from contextlib import ExitStack

    F32 = mybir.dt.float32
    F32R = mybir.dt.float32r
    BF16 = mybir.dt.bfloat16
    AF = mybir.ActivationFunctionType
    Al = mybir.AluOpType

    nc = bass.Bass("TRN2", target_bir_lowering=False, debug=False,
                   num_devices=NCORES)
    img_d = nc.dram_tensor("img", [C, H, W], F32, kind="ExternalInput").ap()
    wr_d = nc.dram_tensor("wr", [128, nr * 128], F32R,
                          kind="ExternalInput").ap()
    wb_d = nc.dram_tensor("wb", [128, 128], BF16, kind="ExternalInput").ap()
    out_d = nc.dram_tensor("out", [H, W], F32, kind="ExternalOutput").ap()

    # weight-load split points: blur bands, then sobel bands, then NMS
    # bands, so slab-0 image loads and the first blur aren't stuck behind
    # one big weight DMA on the serial DMA device.
    n_blur = 1 + max(i for k, i in index.items() if k[0] == "blur")
    n_sob = 1 + max(i for k, i in index.items() if k[0] in ("gx", "gy"))

    def wm(key):
        i = index[key]
        return wrt[:, i * 128:(i + 1) * 128]

    def has(key):
        return key in index

    with tile.TileContext(nc) as tc, ExitStack() as ctx:
        wpool = ctx.enter_context(tc.tile_pool(name="wpool", bufs=1))
        mpool = ctx.enter_context(tc.tile_pool(name="mpool", bufs=5))
        stagep = ctx.enter_context(tc.tile_pool(name="stagep", bufs=1))
        mbp = ctx.enter_context(tc.tile_pool(name="mbp", bufs=3))
        magp = ctx.enter_context(tc.tile_pool(name="magp", bufs=3))
        tl = ctx.enter_context(tc.tile_pool(name="tl", bufs=4))
        ps_mb = ctx.enter_context(
            tc.tile_pool(name="ps_mb", bufs=2, space="PSUM"))
        ps_g = ctx.enter_context(
            tc.tile_pool(name="ps_g", bufs=4, space="PSUM"))
        ps_e = ctx.enter_context(
            tc.tile_pool(name="ps_e", bufs=2, space="PSUM"))

        wrt = wpool.tile([128, nr * 128], F32R, name="wrt")
        wbt = wpool.tile([128, 128], BF16, name="wbt")

        m_of = {}

        def emit_loads(sx, staged=False):
            """Load + channel-sum slab sx into a fresh m tile. Normally c0
            rides SP/HWDGE (plain write) and c1/c2 accumulate via Pool
            SWDGE; `staged` (ramp-up slabs) loads all channels via SP into
            a staging tile and sums on the then-idle DVE, skipping the
            DMA-accumulate dependency chain."""
            if sx >= NSLABS:
                return
            r0x = _r0(sx)
            p_lox = PADL if sx == 0 else 0
            p_hix = (H - 1 - r0x) if sx == NSLABS - 1 else 127
            if sx == 0:
                # ramp fast path (slab 0): keep the 3 channels separate
                # (plain SP loads, no c0->c1->c2 accumulate chain to wait
                # on) and let the blur matmuls sum them (blur-of-sum =
                # sum-of-blurs) on the startup-idle PE.
                mst = m_of[sx] = stagep.tile([128, 3 * FW], F32R,
                                             name="mst", tag="mst")
                for c in range(C):
                    base = c * FW
                    nc.sync.dma_start(
                        mst[p_lox:p_hix + 1, base + PADL:base + PADL + W],
                        img_d[c, r0x + p_lox:r0x + p_hix + 1, :]
                        .bitcast(F32R))
                    if sx == 0:
                        nc.sync.dma_start(
                            mst[PADL - 1:PADL, base + PADL:base + PADL + W],
                            img_d[c, 0:1, :].bitcast(F32R))
                return
            m = m_of[sx] = mpool.tile([128, FW], F32R, name="m", tag="m")
            if sx == NSLABS - 1:
                nc.vector.memset(m[64:128, :].bitcast(F32), 0.0)
            if staged:
                stg = stagep.tile([128, 3 * W], F32, name="stg", tag="stg")
                for c in range(C):
                    nc.sync.dma_start(
                        stg[p_lox:p_hix + 1, c * W:(c + 1) * W],
                        img_d[c, r0x + p_lox:r0x + p_hix + 1, :])
                nc.vector.tensor_tensor(stg[:, 0:W], stg[:, 0:W],
                                        stg[:, W:2 * W], Al.add)
                nc.vector.tensor_tensor(m[:, PADL:PADL + W], stg[:, 0:W],
                                        stg[:, 2 * W:3 * W], Al.add)
            else:
                if sx < 2:
                    nc.sync.dma_start(m[p_lox:p_hix + 1, PADL:PADL + W],
                                      img_d[0, r0x + p_lox:r0x + p_hix + 1,
                                            :].bitcast(F32R))
                else:
                    # later slabs' c0 rides the Pool queue so it cannot
                    # queue-jump slab 0/1's channel sums on the DMA device
                    nc.gpsimd.dma_start(
                        m[p_lox:p_hix + 1, PADL:PADL + W],
                        img_d[0, r0x + p_lox:r0x + p_hix + 1, :])
                for c in range(1, C):
                    nc.gpsimd.dma_start(
                        m[p_lox:p_hix + 1, PADL:PADL + W],
                        img_d[c, r0x + p_lox:r0x + p_hix + 1, :],
                        accum_op=Al.add)
            if sx == NSLABS - 1:             # duplicated bottom edge row
                nc.sync.dma_start(m[p_hix + 1:p_hix + 2, PADL:PADL + W],
                                  img_d[0, H - 1:H, :].bitcast(F32R))
                for c in range(1, C):
                    nc.gpsimd.dma_start(
                        m[p_hix + 1:p_hix + 2, PADL:PADL + W],
                        img_d[c, H - 1:H, :],
                        accum_op=Al.add)

        # prologue: tiny blur-weight DMA early on the ACT HWDGE queue;
        # slabs 0/1 loads; remaining weights behind them on the Pool queue
        # so slab 0's channel sums aren't stuck behind 4us of weights on
        # the FIFO DMA device.
        nc.scalar.dma_start(wrt[:, :n_blur * 128], wr_d[:, :n_blur * 128])
        emit_loads(0)
        emit_loads(1)
        nc.sync.dma_start(wrt[:, n_blur * 128:n_sob * 128],
                          wr_d[:, n_blur * 128:n_sob * 128])
        nc.gpsimd.dma_start(wrt[:, n_sob * 128:], wr_d[:, n_sob * 128:])
        nc.gpsimd.dma_start(wbt[:], wb_d)
        emit_loads(2)
        emit_loads(3)

        for s in range(NSLABS):
            v = 0 if s == 0 else (2 if s == NSLABS - 1 else 1)
            r0 = _r0(s)
            m = m_of.pop(s)
            bases = list(range(0, 3 * FW, FW)) if s == 0 else [0]

            # col edge pads (both columns in one strided op, on ACT)
            for base in bases:
                nc.scalar.activation(
                    m[:, base + PADL - 1:base + PADL + W + 1:W + 1],
                    m[:, base + PADL:base + PADL + W:W - 1].bitcast(F32),
                    AF.Copy)

            # ---- blur (fp32r matmuls straight off m; slab 0 sums its 3
            # separate channel blocks inside the accumulation group) ----
            mb = mbp.tile([128, FW], F32R, name="mb", tag="mb")
            for h in range(2):
                f0 = PADL + CHUNK * h
                pm = ps_mb.tile([128, CHUNK], F32, name="pm", tag="pm")
                mms = [(base, dc) for base in bases for dc in range(3)]
                for i, (base, dc) in enumerate(mms):
                    nc.tensor.matmul(
                        pm[:], wm(("blur", dc)),
                        m[:, base + f0 + dc - 1:base + f0 + dc - 1 + CHUNK],
                        start=(i == 0), stop=(i == len(mms) - 1))
                nc.scalar.activation(mb[:, f0:f0 + CHUNK], pm[:], AF.Copy)
            nc.scalar.activation(mb[:, PADL - 1:PADL + W + 1:W + 1],
                                 mb[:, PADL:PADL + W:W - 1].bitcast(F32),
                                 AF.Copy)

            # ---- sobel + squares ----
            mag = magp.tile([128, FW], F32R, name="mag", tag="mag")
            sqy = tl.tile([128, W], F32, name="sqy", tag="sqy")
            gys = tl.tile([128, W], F32, name="gys", tag="gys")
            gxy = tl.tile([128, W], F32, name="gxy", tag="gxy")
            for h in range(2):
                f0 = PADL + CHUNK * h
                w0 = CHUNK * h
                pgx = ps_g.tile([128, CHUNK], F32, name="pgx", tag="pg")
                mms = [("gx", v, dc) for dc in range(3) if has(("gx", v, dc))]
                for i, key in enumerate(mms):
                    dc = key[2]
                    nc.tensor.matmul(
                        pgx[:], wm(key),
                        mb[:, f0 + dc - 1:f0 + dc - 1 + CHUNK],
                        start=(i == 0), stop=(i == len(mms) - 1))
                pgy = ps_g.tile([128, CHUNK], F32, name="pgy", tag="pg")
                mms = [("gy", v, dc) for dc in range(3) if has(("gy", v, dc))]
                for i, key in enumerate(mms):
                    dc = key[2]
                    nc.tensor.matmul(
                        pgy[:], wm(key),
                        mb[:, f0 + dc - 1:f0 + dc - 1 + CHUNK],
                        start=(i == 0), stop=(i == len(mms) - 1))
                # evacs: sqx straight into mag; sqy; gy copy; gxy
                nc.scalar.activation(mag[:, f0:f0 + CHUNK], pgx[:], AF.Square)
                nc.scalar.activation(sqy[:, w0:w0 + CHUNK], pgy[:], AF.Square)
                nc.scalar.activation(gys[:, w0:w0 + CHUNK], pgy[:], AF.Copy)
                nc.vector.tensor_tensor(
                    gxy[:, w0:w0 + CHUNK], pgx[:], gys[:, w0:w0 + CHUNK],
                    Al.mult)
            # loads for slab s+4: deep lookahead so stores' fin-waits on
            # SP never starve upcoming loads
            emit_loads(s + 4)

            # mag = sqx + sqy in place on DVE: a DMA accumulate queues
            # behind bulk lookahead loads on the FIFO DMA device (+3-7us on
            # the critical chain); the inline add costs 1.1us
            nc.vector.tensor_tensor(mag[:, PADL:PADL + W],
                                    mag[:, PADL:PADL + W].bitcast(F32),
                                    sqy[:], Al.add)
            nc.scalar.activation(mag[:, PADL - 1:PADL + W + 1:W + 1],
                                 mag[:, PADL:PADL + W:W - 1].bitcast(F32),
                                 AF.Copy)

            # ---- orientation: nm = [gxy*(2*sqy - mag) <= lam*mag^2] ----
            magc = mag[:, PADL:PADL + W].bitcast(F32)
            wq = tl.tile([128, W], F32, name="wq", tag="wq")
            cq = tl.tile([128, W], F32, name="cq", tag="cq")
            m2 = tl.tile([128, W], F32, name="m2", tag="m2")
            nm = tl.tile([128, W], BF16, name="nm", tag="nm")
            nc.vector.scalar_tensor_tensor(
                wq[:], sqy[:], 2.0, magc, Al.mult, Al.subtract)
            nc.gpsimd.tensor_tensor(cq[:], gxy[:], wq[:], Al.mult)
            nc.scalar.activation(m2[:], magc, AF.Square, scale=SQLAM)
            nc.vector.tensor_tensor(nm[:], cq[:], m2[:], Al.is_le)

            # ---- NMS: banded shifts (-mag folded), -BIG*nm, z-chain ----
            fin = tl.tile([128, W], F32, name="fin", tag="fin")
            for h in range(2):
                f0 = PADL + CHUNK * h
                w0 = CHUNK * h
                pul = ps_e.tile([128, CHUNK], F32, name="pul", tag="pe")
                mms = [("ul", v, dc) for dc in range(3)
                       if has(("ul", v, dc))]
                for i, key in enumerate(mms):
                    dc = key[2]
                    nc.tensor.matmul(
                        pul[:], wm(key),
                        mag[:, f0 + dc - 1:f0 + dc - 1 + CHUNK],
                        start=(i == 0), stop=False)
                nc.tensor.matmul(pul[:], wbt[:], nm[:, w0:w0 + CHUNK],
                                 start=False, stop=True)
                z1 = tl.tile([128, CHUNK], F32, name="z1", tag="z1")
                nc.vector.scalar_tensor_tensor(
                    z1[:], pul[:], BIGK, mag[:, f0:f0 + CHUNK].bitcast(F32),
                    Al.mult, Al.min)
                pdr = ps_e.tile([128, CHUNK], F32, name="pdr", tag="pe")
                mms = [("dr", v, dc) for dc in range(3)
                       if has(("dr", v, dc))]
                for i, key in enumerate(mms):
                    dc = key[2]
                    nc.tensor.matmul(
                        pdr[:], wm(key),
                        mag[:, f0 + dc - 1:f0 + dc - 1 + CHUNK],
                        start=(i == 0), stop=False)
                nc.tensor.matmul(pdr[:], wbt[:], nm[:, w0:w0 + CHUNK],
                                 start=False, stop=True)
                z2 = tl.tile([128, CHUNK], F32, name="z2", tag="z2")
                nc.vector.scalar_tensor_tensor(
                    z2[:], pdr[:], BIGK, z1[:], Al.mult, Al.min)
                nc.vector.tensor_scalar(
                    fin[:, w0:w0 + CHUNK], z2[:], 0.0, None, Al.max)

            # store on SP; the 4-slab load lookahead keeps c0(s+4..) from
            # being starved behind this store's fin wait.
            row_lo = SLAB * s
            row_hi = min(H - 1, row_lo + SLAB - 1)
            nc.sync.dma_start(out_d[row_lo:row_hi + 1, :],
                              fin[PADL:PADL + row_hi - row_lo + 1, :])

    _split_excess_waits(nc)
    return nc


# ---------------------------------------------------------------------------
def kernel(**inputs):
    _install_fixups()

    img = np.ascontiguousarray(np.asarray(inputs["img"], np.float32))
    gauss_w = np.asarray(inputs["gauss_w"], np.float32)
    sobel_x = np.asarray(inputs["sobel_x"], np.float32)
    sobel_y = np.asarray(inputs["sobel_y"], np.float32)
    dir_w = np.asarray(inputs["dir_w"], np.float32)

    wr, wbig, index = _build_bands(gauss_w, sobel_x, sobel_y, dir_w)
    nr = wr.shape[0]
    # pack bands side-by-side: [128, nr*128]
    wr_pack = np.ascontiguousarray(
        np.concatenate([wr[i] for i in range(nr)], axis=1))
    skey = (_structure_key(index), nr)
    if _CACHE.get("skey") != skey:
        _CACHE["nc"] = _build_module(index, nr)
        _CACHE["skey"] = skey
    nc = _CACHE["nc"]

    from concourse.bass_utils import run_bass_kernel_spmd
    import os
    wbig = np.ascontiguousarray(wbig)
    in_maps = [{"img": np.ascontiguousarray(img[b]), "wr": wr_pack,
                "wb": wbig} for b in range(B)]
    trace = bool(int(os.environ.get("CANNY_TRACE", "0")))
    res = run_bass_kernel_spmd(nc, in_maps, core_ids=list(range(NCORES)),
                               trace=trace)
    if res.exec_time_ns is not None:
        _CACHE["exec_time_ns"] = res.exec_time_ns
    if res.instructions_and_trace is not None:
        _CACHE["trace_path"] = res.instructions_and_trace[1]
    out = np.stack([res.results[b]["out"] for b in range(B)])[:, None]
    return out.astype(np.float32)
